# revision 16
# baseline (speedup 1.0000x reference)
"""Cross-modal attention (B=4, C=512, L=2048, H=8, D=64) on 8 TRN2 NeuronCores.

Sharding: core c handles batch b = c//2 and query-half q = c%2 (1024 queries).
K/V are computed from the full ecg[b] on both cores of a pair (duplicated, no
collectives needed).

v3: the kernel is ACT-bound (softmax exp = 128 ACTIVATE x ~1.1us = 139us of
Scalar-engine time is the floor), so everything else is arranged to hide
under it:
  - fp8e4m3 DoubleRow matmuls (0.5 cycles/row) for the V/K/Q projections,
    the probs@V context matmul and the output projection; only the scores
    matmul (exp argument) and the residual stay bf16.  Weights are host-
    scaled by 16 into fp8's normal range; the 1/16 (resp. 1/256 for the
    doubly-scaled output projection) is folded into the bias add / the
    softmax-denominator scale, so no extra ops are spent.
  - software-pipelined emission: engines execute their streams in order,
    so ctx matmuls (which can block on the previous iteration's softmax
    normalization) are emitted LAG key-blocks behind the score/exp pair,
    and projection / out-projection work is injected into attention loops
    as 'extra' work that fills PE gaps while ACT churns.
  - single persistent PSUM pool: st(scores) 2 banks x2, ctx accumulators
    pc0/pc1 1 bank each, 'mm' (proj/out-proj) 1 bank x2.
  - out = residual + bias accumulated in SBUF f32; per-tile DMA out.
"""

import os
import numpy as np

B = 4
C = 512
L = 2048
H = 8
D = 64
LQ = 1024          # queries per core
P = 128
NCB = C // P       # 4 c-blocks (also head-pairs)
NKB = L // P       # 16 key blocks of 128
NG = NKB // 2      # 8 key groups of 256 (fp8 DoubleRow ctx)

_CACHED = {}


def _build():
    import concourse.tile as tile
    from concourse import bacc, mybir

    F32 = mybir.dt.float32
    CDT = mybir.dt.bfloat16
    F8 = mybir.dt.float8e4
    EXP = mybir.ActivationFunctionType.Exp
    DR = mybir.MatmulPerfMode.DoubleRow
    MUL = mybir.AluOpType.mult
    ADD = mybir.AluOpType.add

    nc = bacc.Bacc("TRN2", target_bir_lowering=False, debug=False)

    ppg_c8 = nc.dram_tensor("ppg_c8", (C, LQ), F8, kind="ExternalInput").ap()
    ppg_cb = nc.dram_tensor("ppg_cb", (C, LQ), CDT, kind="ExternalInput").ap()
    ecg_b8 = nc.dram_tensor("ecg_b8", (C, L), F8, kind="ExternalInput").ap()
    wqt8h = nc.dram_tensor("wqt8", (C, C), F8, kind="ExternalInput").ap()
    wkt8h = nc.dram_tensor("wkt8", (C, C), F8, kind="ExternalInput").ap()
    wvt8h = nc.dram_tensor("wvt8", (C, C), F8, kind="ExternalInput").ap()
    wot8h = nc.dram_tensor("wot8", (64, NCB, 2, C), F8,
                           kind="ExternalInput").ap()
    bq = nc.dram_tensor("bq", (C,), F32, kind="ExternalInput").ap()
    bk = nc.dram_tensor("bk", (C,), F32, kind="ExternalInput").ap()
    bv16 = nc.dram_tensor("bv16", (C,), F32, kind="ExternalInput").ap()
    bo = nc.dram_tensor("bo", (C,), F32, kind="ExternalInput").ap()
    outp = nc.dram_tensor("outp", (C, LQ), F32, kind="ExternalOutput").ap()

    with tile.TileContext(nc) as tc:
        with (
            tc.tile_pool(name="persist", bufs=1) as persist,
            tc.tile_pool(name="psum", bufs=1, space="PSUM") as psum,
            tc.tile_pool(name="exp_pool", bufs=6) as exp_pool,
            tc.tile_pool(name="sm_pool", bufs=2) as sm_pool,
        ):
            # ---- input DMAs, ordered by first use, spread over 3 queues ----
            ecg8 = persist.tile([P, NCB, L], F8)
            wkt8 = persist.tile([P, NCB, C], F8)
            wvt8 = persist.tile([P, NCB, C], F8)
            wqt8 = persist.tile([P, NCB, C], F8)
            ppg8 = persist.tile([P, NCB, LQ], F8)
            ppg_c = persist.tile([P, NCB, LQ], CDT)
            bq_t = persist.tile([P, NCB], F32)
            bk_t = persist.tile([P, NCB], F32)
            bo_t = persist.tile([P, NCB], F32)
            bv_row = persist.tile([1, C], CDT)
            wot8_t = persist.tile([64, NCB, 2, C], F8)
            ecg_hbm = ecg_b8.rearrange("(s p) l -> p s l", p=P)
            wkt_hbm = wkt8h.rearrange("(s p) o -> p s o", p=P)
            nc.gpsimd.dma_start(wkt8[:, :, 0:P], wkt_hbm[:, :, 0:P])
            nc.sync.dma_start(ecg8[:, 0, :], ecg_hbm[:, 0, :])
            nc.scalar.dma_start(ecg8[:, 2, :], ecg_hbm[:, 2, :])
            nc.sync.dma_start(ecg8[:, 1, :], ecg_hbm[:, 1, :])
            nc.gpsimd.dma_start(wvt8[:], wvt8h.rearrange("(s p) o -> p s o", p=P))
            nc.gpsimd.dma_start(ecg8[:, 3, :], ecg_hbm[:, 3, :])
            nc.gpsimd.dma_start(bv_row[0:1, :], bv16[None, :])
            nc.sync.dma_start(bk_t[:], bk.rearrange("(s p) -> p s", p=P))
            nc.sync.dma_start(bq_t[:], bq.rearrange("(s p) -> p s", p=P))
            nc.sync.dma_start(bo_t[:], bo.rearrange("(s p) -> p s", p=P))
            nc.scalar.dma_start(ppg8[:], ppg_c8.rearrange("(s p) l -> p s l", p=P))
            nc.gpsimd.dma_start(wkt8[:, :, P:], wkt_hbm[:, :, P:])
            nc.gpsimd.dma_start(wqt8[:], wqt8h.rearrange("(s p) o -> p s o", p=P))
            nc.sync.dma_start(ppg_c[:], ppg_cb.rearrange("(s p) l -> p s l", p=P))
            nc.gpsimd.dma_start(wot8_t[:], wot8h)

            ones_t = persist.tile([1, P], CDT)
            nc.vector.memset(ones_t[:], 1.0)
            ones_col = persist.tile([P, 1], F32)
            nc.vector.memset(ones_col[:], 1.0)

            # ---- persistent activations ----
            qT = persist.tile([P, NCB, LQ], CDT)
            kT = persist.tile([P, NCB, L], CDT)
            # v8: 16*(y@Wv^T+bv) in fp8, key-group-paired for DoubleRow,
            # padded M 65->80 (dual-fp8 ldweights needs 16B-aligned step),
            # ones column at D for the softmax denominator row.
            v8 = persist.tile([P, NG, 2, H, 80], F8)
            ctxT8 = persist.tile([64, NCB, 2, LQ], F8)   # 16*ctx/den
            out_acc = persist.tile([P, NCB, LQ], F32)
            nc.vector.tensor_copy(
                out=v8[:, :, :, :, D:D + 1],
                in_=ones_col[:, None, None, None, :].to_broadcast(
                    (P, NG, 2, H, 1)))

            def kT_chunk(cb, kb5):
                # kT[:, cb, 512-chunk] = (16*Wk @ y^T)/16 + bk
                ps_k = psum.tile([P, 512], F32, tag="mm", bufs=2)
                for s2 in range(2):
                    nc.tensor.matmul(
                        ps_k[:], wkt8[:, 2 * s2:2 * s2 + 2, cb * P:(cb + 1) * P],
                        ecg8[:, 2 * s2:2 * s2 + 2, kb5 * 512:(kb5 + 1) * 512],
                        start=(s2 == 0), stop=(s2 == 1), perf_mode=DR)
                nc.vector.tensor_scalar(
                    out=kT[:, cb, kb5 * 512:(kb5 + 1) * 512], in0=ps_k[:],
                    scalar1=1.0 / 16.0, scalar2=bk_t[:, cb:cb + 1],
                    op0=MUL, op1=ADD)

            def qT_chunk(cb, qb5):
                ps_q = psum.tile([P, 512], F32, tag="mm", bufs=2)
                for s2 in range(2):
                    nc.tensor.matmul(
                        ps_q[:], wqt8[:, 2 * s2:2 * s2 + 2, cb * P:(cb + 1) * P],
                        ppg8[:, 2 * s2:2 * s2 + 2, qb5 * 512:(qb5 + 1) * 512],
                        start=(s2 == 0), stop=(s2 == 1), perf_mode=DR)
                nc.vector.tensor_scalar(
                    out=qT[:, cb, qb5 * 512:(qb5 + 1) * 512], in0=ps_q[:],
                    scalar1=1.0 / 16.0, scalar2=bq_t[:, cb:cb + 1],
                    op0=MUL, op1=ADD)

            def v_block(lb):
                # v8[lb] = 16*(y[lb] @ Wv^T + bv)  (head-strided)
                ps_v = psum.tile([P, 512], F32, tag="mm", bufs=2)
                for s2 in range(2):
                    nc.tensor.matmul(
                        ps_v[:], ecg8[:, 2 * s2:2 * s2 + 2, lb * P:(lb + 1) * P],
                        wvt8[:, 2 * s2:2 * s2 + 2, :],
                        start=(s2 == 0), stop=False, perf_mode=DR)
                nc.tensor.matmul(ps_v[:], ones_t[0:1, :], bv_row[0:1, :],
                                 start=False, stop=True)
                nc.vector.tensor_copy(
                    out=v8[:, lb // 2, lb % 2, :, 0:D],
                    in_=ps_v[:].rearrange("p (h d) -> p h d", d=D))

            LAG = 6

            def attn(pair, qb, extra=None):
                # scores/exp per 128-key block; fp8 DoubleRow ctx per
                # 256-key group, emitted LAG blocks later so the (in-order)
                # PE stream never stalls ACT behind a norm-blocked ctx.
                qsl = slice(qb * 512, (qb + 1) * 512)
                pc0 = psum.tile([P, 512], F32, tag="pc0", bufs=1)
                pc1 = psum.tile([P, 512], F32, tag="pc1", bufs=1)
                pcs = (pc0, pc1)
                e8s = {}
                for kb in range(NKB + LAG):
                    if kb < NKB:
                        g, t = kb // 2, kb % 2
                        if t == 0:
                            e8s[g] = exp_pool.tile([P, 2, 2, 512], F8,
                                                   name="e8t", tag="e8",
                                                   bufs=6)
                        st = psum.tile([P, 2, 512], F32, tag="st", bufs=2)
                        for hl in range(2):
                            nc.tensor.matmul(
                                st[:, hl, :],
                                kT[64 * hl:64 * hl + 64, pair,
                                   kb * P:(kb + 1) * P],
                                qT[64 * hl:64 * hl + 64, pair, qsl],
                                start=True, stop=True)
                        nc.scalar.activation(e8s[g][:, t, :, :], st[:],
                                             EXP, scale=0.125)
                        if extra is not None and kb in extra:
                            extra[kb]()
                    j = kb - LAG
                    if j >= 1 and j % 2 == 1:
                        g = j // 2
                        for hl in range(2):
                            nc.tensor.matmul(
                                pcs[hl][0:D + 1, :],
                                v8[:, g, :, 2 * pair + hl, 0:D + 1],
                                e8s[g][:, :, hl, :],
                                start=(g == 0), stop=(g == NG - 1),
                                perf_mode=DR)
                        if g >= 1:
                            del e8s[g - 1]
                for hl in range(2):
                    den = sm_pool.tile([1, 512], F32)
                    nc.vector.tensor_copy(out=den[0:1, :],
                                          in_=pcs[hl][D:D + 1, :])
                    recip = sm_pool.tile([1, 512], F32)
                    nc.vector.reciprocal_approx_fast(
                        out=recip[0:1, :], in_=den[0:1, :])
                    rbc = sm_pool.tile([64, 512], F32)
                    nc.gpsimd.partition_broadcast(rbc[:], recip[0:1, :],
                                                  channels=64)
                    nc.vector.tensor_mul(
                        out=ctxT8[:, pair, hl, qsl], in0=pcs[hl][0:D, :],
                        in1=rbc[:])

            def po_pair(qb, cb, pair):
                # one DoubleRow matmul = this head-pair's contribution to
                # out[cb, qb]; accumulated straight into SBUF so no PSUM
                # bank is held across attention iterations.
                qsl = slice(qb * 512, (qb + 1) * 512)
                po = psum.tile([P, 512], F32, tag="mm", bufs=2)
                nc.tensor.matmul(
                    po[:], wot8_t[:, pair, :, cb * P:(cb + 1) * P],
                    ctxT8[:, pair, :, qsl],
                    start=True, stop=True, perf_mode=DR)
                # out_acc += po/256  (16*Wo and 16*ctx scaling)
                nc.vector.scalar_tensor_tensor(
                    out=out_acc[:, cb, qsl], in0=po[:],
                    scalar=1.0 / 256.0, in1=out_acc[:, cb, qsl],
                    op0=MUL, op1=ADD)
                if pair == H // 2 - 1:
                    nc.sync.dma_start(
                        outp.rearrange("(s p) l -> p s l", p=P)[:, cb, qsl],
                        out_acc[:, cb, qsl])

            # ---- pipelined emission ----
            kT_chunk(0, 0)
            qT_chunk(0, 0)
            v_block(0)
            # out_acc = residual + output bias (DVE fills gaps early)
            for cb in range(NCB):
                for qb in range(LQ // 512):
                    qsl = slice(qb * 512, (qb + 1) * 512)
                    nc.vector.tensor_scalar_add(
                        out_acc[:, cb, qsl], ppg_c[:, cb, qsl],
                        bo_t[:, cb:cb + 1])

            def merge(*exs):
                out = {}
                for ex in exs:
                    for k, fn in ex.items():
                        if k in out:
                            out[k] = (lambda a=out[k], b=fn: (a(), b()))
                        else:
                            out[k] = fn
                return out

            def proj_extras(cb):
                ex = {}
                for k5 in range(4):
                    ex[4 * k5] = (lambda c=cb, k=k5: kT_chunk(c, k))
                ex[2] = (lambda c=cb: qT_chunk(c, 0))
                ex[6] = (lambda c=cb: qT_chunk(c, 1))
                return ex

            def po_extras(qb, pair):
                # injected into the attention iteration FOLLOWING the norm
                # that produces ctxT8[pair, qb]; slots >= 7 so the in-order
                # PE stream never reaches them before the norm completes.
                return {7 + 2 * cb: (lambda q=qb, c=cb, p=pair: po_pair(q, c, p))
                        for cb in range(NCB)}

            ex00 = {kb: (lambda lb=kb + 1: v_block(lb))
                    for kb in range(NKB - 1)}
            ex00 = merge(ex00,
                         {0: (lambda: kT_chunk(0, 1)),
                          4: (lambda: kT_chunk(0, 2)),
                          8: (lambda: kT_chunk(0, 3)),
                          2: (lambda: qT_chunk(0, 1))})
            attn(0, 0, extra=ex00)
            attn(0, 1, extra=merge(proj_extras(1), po_extras(0, 0)))
            attn(1, 0, extra=po_extras(1, 0))
            attn(1, 1, extra=merge(proj_extras(2), po_extras(0, 1)))
            attn(2, 0, extra=po_extras(1, 1))
            attn(2, 1, extra=merge(proj_extras(3), po_extras(0, 2)))
            attn(3, 0, extra=po_extras(1, 2))
            attn(3, 1, extra=po_extras(0, 3))
            for cb in range(NCB):
                po_pair(1, cb, 3)
    nc.compile()
    return nc


def _get_nc():
    if "nc" not in _CACHED:
        _CACHED["nc"] = _build()
    return _CACHED["nc"]


def kernel(ppg, ecg, Wq, bq, Wk, bk, Wv, bv, Wo, bo):
    import ml_dtypes
    from concourse.bass_utils import run_bass_kernel_spmd

    nc = _get_nc()
    f = np.float32
    bf = ml_dtypes.bfloat16
    f8 = ml_dtypes.float8_e4m3fn
    wqt8 = np.ascontiguousarray((np.asarray(Wq, f).T * 16).astype(f8))
    wkt8 = np.ascontiguousarray((np.asarray(Wk, f).T * 16).astype(f8))
    wvt8 = np.ascontiguousarray((np.asarray(Wv, f).T * 16).astype(f8))
    # wot8[d, p, hl, o] = 16 * Wo[o, (2p+hl)*64 + d]
    wot8 = np.ascontiguousarray(
        (np.asarray(Wo, f).T * 16).reshape(NCB, 2, D, C)
        .transpose(2, 0, 1, 3).astype(f8))
    ppg = np.asarray(ppg, f)
    ecg = np.asarray(ecg, f)
    in_maps = []
    for c in range(8):
        b, half = c // 2, c % 2
        ppg_b = ppg[b][:, half * LQ:(half + 1) * LQ]
        in_maps.append({
            "ppg_c8": np.ascontiguousarray(ppg_b.astype(f8)),
            "ppg_cb": np.ascontiguousarray(ppg_b.astype(bf)),
            "ecg_b8": np.ascontiguousarray(ecg[b].astype(f8)),
            "wqt8": wqt8, "wkt8": wkt8, "wvt8": wvt8, "wot8": wot8,
            "bq": np.asarray(bq, f), "bk": np.asarray(bk, f),
            "bv16": np.asarray(bv, f) * 16, "bo": np.asarray(bo, f),
        })
    _CACHED["last_in_maps"] = in_maps
    res = run_bass_kernel_spmd(nc, in_maps, core_ids=list(range(8)))
    out = np.empty((B, C, L), f)
    for c, r in enumerate(res.results):
        b, half = c // 2, c % 2
        out[b][:, half * LQ:(half + 1) * LQ] = r["outp"]
    return out
